# revision 1
# baseline (speedup 1.0000x reference)
"""MoE expert-parallel kernel for Trainium2 (Bass/Tile).

8 experts, 8 NeuronCores, one expert per core (SPMD, no collectives).
Per core: out = gelu(x @ w1) @ w2 with
  x  [2048, 1024] f32, w1 [1024, 4096] f32, w2 [4096, 1024] f32.

Matmuls run as float32r (TF32-class PE fast path, fp32 storage): rel err vs
the fp32 reference ~2e-4.

Structure per core — two token-halves of 1024, each:
  Phase 0: DMA x row-tiles, PE-transpose to xT [k=128p, 8, 1024] in SBUF.
  For each of 8 n-blocks (512 wide) of the intermediate dim:
    - stream w1[:, blk] and w2[blk, :] from HBM
    - GEMM1 (f32r, free dim 512): psum[n128, tok512] over 8 k-tiles
    - GELU eviction on ACT: psum -> hT_blk [n128p, 4, 1024] SBUF (rounds f32r)
    - GEMM2 (f32r): psum[tok128, d512] over the block's 4 n-tiles,
      DVE-accumulated into out_acc [tok128p, 8, 1024] f32 SBUF
  After the half's last block: DMA out row-tiles to HBM.
"""

import os
import sys

import numpy as np

if os.path.isdir("/opt/trn_rl_repo") and "/opt/trn_rl_repo" not in sys.path:
    sys.path.insert(0, "/opt/trn_rl_repo")

# ---------------------------------------------------------------------------
# Workaround for walrus codegen "Too many sync wait commands": this compiler
# build encodes at most 1 sem-wait per instruction. Split excess on_wait
# entries onto NoOp instructions inserted immediately before the offender on
# the same engine — identical semantics, since an engine executes its stream
# in order.


def _split_excess_waits(bir_json: bytes) -> bytes:
    import orjson

    d = orjson.loads(bir_json)
    changed = False
    for fn in d.get("functions", []):
        for blk in fn.get("blocks", []):
            out = []
            for ins in blk.get("instructions", []):
                sync = ins.get("sync_info")
                waits = (sync or {}).get("on_wait") or []
                cap = 1
                if len(waits) > cap:
                    changed = True
                    extra, keep = waits[:-cap], waits[-cap:]
                    for i in range(len(extra)):
                        out.append({
                            "name": f"{ins['name']}-wsplit{i}",
                            "opcode": "NoOp",
                            "engine": ins["engine"],
                            "ins": [],
                            "outs": [],
                            "debug": ins.get("debug", 0),
                            "sync_info": {"on_update": [], "on_wait": [extra[i]]},
                        })
                    sync["on_wait"] = keep
                out.append(ins)
            blk["instructions"] = out
    return orjson.dumps(d) if changed else bir_json


_hook_installed = False


def _install_wait_split_hook():
    global _hook_installed
    if _hook_installed:
        return
    import concourse.bass2jax as bass2jax
    import concourse.bass_utils as bass_utils

    orig = bass_utils.compile_bir_kernel

    def patched(bir_json, tmpdir, neff_name="file.neff"):
        return orig(_split_excess_waits(bir_json), tmpdir, neff_name)

    bass2jax.compile_bir_kernel = patched
    bass_utils.compile_bir_kernel = patched
    _hook_installed = True


NUM_EXPERTS = 8
TOK = 2048
HID = 1024
INT = 4096
OUT = 1024
P = 128

TH = 1024            # token half
NTH = TOK // TH      # 2 halves
NB = 512             # n-block width (intermediate dim)
NBLK = INT // NB     # 8 blocks
NT = NB // P         # 4 n-tiles per block
TCH = 512            # tok chunk (GEMM1 moving free dim)
NCH = TH // TCH      # 2 chunks per half
KT = HID // P        # 8 k-tiles for GEMM1
TT = TH // P         # 8 tok tiles per half
ND = OUT // 512      # 2 d-halves for GEMM2 moving dim

_PROGRAM_CACHE = {}


def build_program(mm_dtype_name=None, repeats=1):
    """Build the per-core Bass program. Returns the finalized Bass object.

    repeats>1 re-emits the whole kernel body that many times in one program
    (used only for timing-by-amplification in test.py).
    """
    import concourse.bass as bass
    import concourse.tile as tile
    from concourse import mybir
    from concourse.masks import make_identity

    f32 = mybir.dt.float32
    if mm_dtype_name is None:
        mm_dtype_name = os.environ.get("MOE_MM_DTYPE", "float32r")
    if mm_dtype_name == "mixed":
        # GEMM1 in f32r (precision), GEMM2 in bf16 (speed)
        g1_dt = mybir.dt.float32r
        g2_dt = mybir.dt.bfloat16
    else:
        g1_dt = g2_dt = getattr(mybir.dt, mm_dtype_name)

    nc = bass.Bass()
    x_h = nc.declare_dram_parameter("x", [TOK, HID], f32, isOutput=False)
    w1_h = nc.declare_dram_parameter("w1", [HID, INT], f32, isOutput=False)
    w2_h = nc.declare_dram_parameter("w2", [INT, OUT], f32, isOutput=False)
    out_h = nc.declare_dram_parameter("out", [TOK, OUT], f32, isOutput=True)

    x_ap = x_h[:, :]
    out_ap = out_h[:, :]
    # w1 [k, n] -> [p, a, n] with k = a*128 + p  (partition = k within tile)
    w1r = w1_h[:, :].rearrange("(a p) n -> p a n", p=P)
    # w2 [n, d] -> [p, a, d] with n = a*128 + p
    w2r = w2_h[:, :].rearrange("(a p) d -> p a d", p=P)

    gelu = getattr(
        mybir.ActivationFunctionType, os.environ.get("MOE_ACT", "Gelu")
    )

    with tile.TileContext(nc) as tc:
        with (
            tc.tile_pool(name="const", bufs=1) as const_pool,
            tc.tile_pool(name="xrow", bufs=2) as xrow_pool,
            tc.tile_pool(name="tpsum", bufs=2, space="PSUM") as tpsum_pool,
            tc.tile_pool(name="xtp", bufs=1) as xt_pool,
            tc.tile_pool(name="w1p", bufs=2) as w1_pool,
            tc.tile_pool(name="w2p", bufs=3) as w2_pool,
            tc.tile_pool(name="htp", bufs=2) as ht_pool,
            tc.tile_pool(name="hpsum", bufs=3, space="PSUM") as hpsum_pool,
            tc.tile_pool(name="opsum", bufs=3, space="PSUM") as opsum_pool,
            tc.tile_pool(name="outp", bufs=1) as out_pool,
        ):
            ident = const_pool.tile([P, P], f32, name="ident")
            make_identity(nc, ident)

            pools = (xt_pool, out_pool, xrow_pool, tpsum_pool, w1_pool,
                     w2_pool, ht_pool, hpsum_pool, opsum_pool)
            for _rep in range(repeats):
                for half in range(NTH):
                    _emit_half(nc, tc, half, g1_dt, g2_dt, f32, gelu,
                               x_ap, out_ap, w1r, w2r, pools, ident)

    return nc


def _emit_half(nc, tc, half, g1_dt, g2_dt, f32, gelu, x_ap, out_ap,
               w1r, w2r, pools, ident):
    (xt_pool, out_pool, xrow_pool, tpsum_pool, w1_pool, w2_pool, ht_pool,
     hpsum_pool, opsum_pool) = pools
    from concourse import mybir
    t0 = half * TH  # first token of this half

    def wdma(out_tile, in_ap_f32, wdt):
        # weight load: HWDGE bitcast for 4-byte matmul dtypes (same bits),
        # SWDGE casting DMA for 2-byte dtypes (gpsimd converts f32->bf16)
        if mybir.dt.size(wdt) == 2:
            nc.gpsimd.dma_start(out=out_tile, in_=in_ap_f32)
        elif wdt is f32:
            nc.sync.dma_start(out=out_tile, in_=in_ap_f32)
        else:
            nc.sync.dma_start(out=out_tile, in_=in_ap_f32.bitcast(wdt))

    xt = xt_pool.tile([P, KT, TH], g1_dt, name="xt")
    out_acc = out_pool.tile([P, TT, OUT], f32, name="out_acc")

    def transpose_chunk(c):
        # transpose the 4 x row-tiles backing tok chunk c into xt
        for r in range(4 * c, 4 * (c + 1)):
            xrow = xrow_pool.tile([P, HID], f32, name="xrow")
            nc.sync.dma_start(
                out=xrow[:], in_=x_ap[t0 + r * P:t0 + (r + 1) * P, :]
            )
            for k in range(KT):
                tp = tpsum_pool.tile([P, P], f32, name="tp")
                nc.tensor.transpose(
                    tp[:], xrow[:, k * P:(k + 1) * P], ident[:]
                )
                nc.scalar.copy(xt[:, k, r * P:(r + 1) * P], tp[:])

    def emit_gemm2(b, htb, w2b):
        # GEMM2: out[tok, d] += hT[:, tok].T @ w2[blk, d]
        for t in range(TT):
            for d in range(ND):
                po = opsum_pool.tile([P, 512], f32, name="po")
                for j in range(NT):
                    nc.tensor.matmul(
                        po[:],
                        htb[:, j, t * P:(t + 1) * P],
                        w2b[:, j, d * 512:(d + 1) * 512],
                        start=(j == 0),
                        stop=(j == NT - 1),
                    )
                if b == 0:
                    nc.vector.tensor_copy(
                        out_acc[:, t, d * 512:(d + 1) * 512], po[:]
                    )
                else:
                    nc.vector.tensor_add(
                        out_acc[:, t, d * 512:(d + 1) * 512],
                        out_acc[:, t, d * 512:(d + 1) * 512],
                        po[:],
                    )
            if b == NBLK - 1:
                nc.sync.dma_start(
                    out=out_ap[t0 + t * P:t0 + (t + 1) * P, :],
                    in_=out_acc[:, t, :],
                )

    # ---- Main loop over n-blocks, one-block software pipeline ---------
    # PE order: G1(0), G1(1), G2(0), G1(2), G2(1), ..., G1(7), G2(6), G2(7)
    # so GEMM2(b) runs a full block after its GELU evictions — PE never
    # waits on ACT at block boundaries. htb/w2b pools have bufs=2.
    prev = None
    for b in range(NBLK):
        w1b = w1_pool.tile([P, KT, NB], g1_dt, name="w1b")
        w2b = w2_pool.tile([P, NT, OUT], g2_dt, name="w2b")
        for j in range(NT):
            # split weight DMAs by n-tile so GEMM1 group j can start as
            # soon as its own slice has landed
            wdma(w1b[:, :, j * P:(j + 1) * P],
                 w1r[:, :, b * NB + j * P:b * NB + (j + 1) * P], g1_dt)
            wdma(w2b[:, j, :], w2r[:, b * NT + j, :], g2_dt)

        htb = ht_pool.tile([P, NT, TH], g2_dt, name="htb")

        # GEMM1: hT[n, tok] = w1[:, n].T @ xT ; GELU into htb
        # chunk-major so the half's first transposes overlap block 0's GEMM1
        for c in range(NCH):
            if b == 0:
                transpose_chunk(c)
            for j in range(NT):
                ph = hpsum_pool.tile([P, TCH], f32, name="ph")
                for k in range(KT):
                    nc.tensor.matmul(
                        ph[:],
                        w1b[:, k, j * P:(j + 1) * P],
                        xt[:, k, c * TCH:(c + 1) * TCH],
                        start=(k == 0),
                        stop=(k == KT - 1),
                    )
                nc.scalar.activation(
                    htb[:, j, c * TCH:(c + 1) * TCH], ph[:], gelu
                )

        if prev is not None:
            emit_gemm2(*prev)
        prev = (b, htb, w2b)
    emit_gemm2(*prev)


def _get_program():
    key = os.environ.get("MOE_MM_DTYPE", "float32r")
    if key not in _PROGRAM_CACHE:
        _PROGRAM_CACHE[key] = build_program(key)
    return _PROGRAM_CACHE[key]


def kernel(x, w1, w2, _trace=False, _trace_kwargs=None):
    """Full-tensor entry point: shards experts across 8 cores, returns full out."""
    from concourse.bass_utils import run_bass_kernel_spmd

    _install_wait_split_hook()
    x = np.ascontiguousarray(x, dtype=np.float32)
    w1 = np.ascontiguousarray(w1, dtype=np.float32)
    w2 = np.ascontiguousarray(w2, dtype=np.float32)
    assert x.shape == (NUM_EXPERTS, TOK, HID)
    assert w1.shape == (NUM_EXPERTS, HID, INT)
    assert w2.shape == (NUM_EXPERTS, INT, OUT)

    nc = _get_program()
    core_ids = list(range(NUM_EXPERTS))
    in_maps = [
        {"x": x[e], "w1": w1[e], "w2": w2[e]} for e in range(NUM_EXPERTS)
    ]
    kw = {}
    if _trace:
        kw["trace"] = True
        kw["trace_kwargs"] = _trace_kwargs or {}
    res = run_bass_kernel_spmd(nc, in_maps, core_ids, **kw)
    out = np.stack([res.results[e]["out"] for e in range(NUM_EXPERTS)], axis=0)
    if _trace:
        return out, res
    return out


if __name__ == "__main__":
    rng = np.random.default_rng(0)
    x = rng.standard_normal((NUM_EXPERTS, TOK, HID), dtype=np.float32)
    w1 = rng.standard_normal((NUM_EXPERTS, HID, INT), dtype=np.float32) * 0.03
    w2 = rng.standard_normal((NUM_EXPERTS, INT, OUT), dtype=np.float32) * 0.015
    out = kernel(x, w1, w2)
    print("out", out.shape, out.dtype, float(np.abs(out).mean()))



# revision 2
# speedup vs baseline: 1.0615x; 1.0615x over previous
"""MoE expert-parallel kernel v3 for Trainium2 (Bass/Tile).

8 experts, 8 NeuronCores, one expert per core (SPMD, no collectives).
Per core: out = gelu(x @ w1) @ w2 with
  x [2048, 1024] f32, w1 [1024, 4096] f32, w2 [4096, 1024] f32.

v3 design (vs baseline): all matmuls in bf16 (pure-stream ~193 ns/MM vs
f32r's ~222), PE stream is matmuls ONLY:
  - w1, w2 fully SBUF-resident as bf16 (64+64 KB/partition), loaded once
    per rep via gpsimd casting DMAs (f32 HBM -> bf16 SBUF).
  - x transposed via DMA XBAR transpose (SBUF bf16 -> SBUF), not the PE.
  - 4 token stripes of 512; per stripe: GEMM1 (32 chains x 8 MMs, N=512)
    with GELU eviction to bf16 hT, then GEMM2 as 8 chains of 32 MMs
    PSUM-accumulated across the whole 4096 contraction -> one ACT
    eviction + DMA per chain (no DVE block-accumulation).
  - hT split lo/hi so GEMM2 chain starts don't wait on the last GELU.
"""

import os
import sys

import numpy as np

if os.path.isdir("/opt/trn_rl_repo") and "/opt/trn_rl_repo" not in sys.path:
    sys.path.insert(0, "/opt/trn_rl_repo")

# ---------------------------------------------------------------------------
# Workaround for walrus codegen "Too many sync wait commands": this compiler
# build encodes at most 1 sem-wait per instruction. Split excess on_wait
# entries onto NoOp instructions inserted immediately before the offender on
# the same engine — identical semantics, since an engine executes its stream
# in order.


def _split_excess_waits(bir_json: bytes) -> bytes:
    import orjson

    d = orjson.loads(bir_json)
    changed = False
    for fn in d.get("functions", []):
        for blk in fn.get("blocks", []):
            out = []
            for ins in blk.get("instructions", []):
                sync = ins.get("sync_info")
                waits = (sync or {}).get("on_wait") or []
                cap = 1
                if len(waits) > cap:
                    changed = True
                    extra, keep = waits[:-cap], waits[-cap:]
                    for i in range(len(extra)):
                        out.append({
                            "name": f"{ins['name']}-wsplit{i}",
                            "opcode": "NoOp",
                            "engine": ins["engine"],
                            "ins": [],
                            "outs": [],
                            "debug": ins.get("debug", 0),
                            "sync_info": {"on_update": [], "on_wait": [extra[i]]},
                        })
                    sync["on_wait"] = keep
                out.append(ins)
            blk["instructions"] = out
    return orjson.dumps(d) if changed else bir_json


_hook_installed = False


def _install_wait_split_hook():
    global _hook_installed
    if _hook_installed:
        return
    import concourse.bass2jax as bass2jax
    import concourse.bass_utils as bass_utils

    orig = bass_utils.compile_bir_kernel

    def patched(bir_json, tmpdir, neff_name="file.neff"):
        return orig(_split_excess_waits(bir_json), tmpdir, neff_name)

    bass2jax.compile_bir_kernel = patched
    bass_utils.compile_bir_kernel = patched
    _hook_installed = True


NUM_EXPERTS = 8
TOK = 2048
HID = 1024
INT = 4096
OUT = 1024
P = 128

TH = 512             # tokens per stripe
NS = TOK // TH       # 4 stripes
KT = HID // P        # 8 k-tiles (GEMM1 contraction)
NT1 = INT // P       # 32 n-tiles (intermediate)
TT = TH // P         # 4 token tiles per stripe
ND = OUT // 512      # 2 d-halves (GEMM2 moving spans)
RT = TH // P         # 4 x row-tiles per stripe

W1C = 16             # w1 load chunks (n-width 256 each)
W2C = 16             # w2 load chunks (2 n-tiles each)

_PROGRAM_CACHE = {}


def build_program(key=None, repeats=1, ablate=None):
    import concourse.bass as bass
    import concourse.tile as tile
    from concourse import mybir

    if ablate is None:
        ablate = ""
    ablate = set(a for a in ablate.split(",") if a)

    f32 = mybir.dt.float32
    bf16 = mybir.dt.bfloat16

    nc = bass.Bass()
    x_h = nc.declare_dram_parameter("x", [TOK, HID], f32, isOutput=False)
    w1_h = nc.declare_dram_parameter("w1", [HID, INT], f32, isOutput=False)
    w2_h = nc.declare_dram_parameter("w2", [INT, OUT], f32, isOutput=False)
    out_h = nc.declare_dram_parameter("out", [TOK, OUT], f32, isOutput=True)

    x_ap = x_h[:, :]
    out_ap = out_h[:, :]
    # w1 [k, n] -> [p, a, n] with k = a*128 + p (partition = k within tile)
    w1r = w1_h[:, :].rearrange("(a p) n -> p a n", p=P)
    # w2 [n, d] -> [p, a, d] with n = a*128 + p
    w2r = w2_h[:, :].rearrange("(a p) d -> p a d", p=P)

    gelu = getattr(
        mybir.ActivationFunctionType, "Gelu"
    )

    with tile.TileContext(nc) as tc:
        with (
            tc.tile_pool(name="w1p", bufs=1) as w1_pool,
            tc.tile_pool(name="w2p", bufs=1) as w2_pool,
            tc.tile_pool(name="xrow", bufs=8) as xrow_pool,
            tc.tile_pool(name="xtp", bufs=4 if "nox" in ablate else 2) as xt_pool,
            tc.tile_pool(name="htp", bufs=1) as ht_pool,
            tc.tile_pool(name="ostg", bufs=4) as ost_pool,
            tc.tile_pool(name="hpsum", bufs=3, space="PSUM") as hpsum_pool,
            tc.tile_pool(name="opsum", bufs=2, space="PSUM") as opsum_pool,
        ):
            pools = (w1_pool, w2_pool, xrow_pool, xt_pool, ht_pool,
                     ost_pool, hpsum_pool, opsum_pool)
            pre = {}
            if "noreload" in ablate or "nog1" in ablate:
                w1t = w1_pool.tile([P, KT, INT], bf16, name="w1t")
                w2t = w2_pool.tile([P, NT1, OUT], bf16, name="w2t")
                cw = INT // W1C
                if "nog1" not in ablate:
                    for c in range(W1C):
                        nc.gpsimd.dma_start(
                            out=w1t[:, :, c * cw:(c + 1) * cw],
                            in_=w1r[:, :, c * cw:(c + 1) * cw])
                ca = NT1 // W2C
                for c in range(W2C):
                    nc.gpsimd.dma_start(
                        out=w2t[:, c * ca:(c + 1) * ca, :],
                        in_=w2r[:, c * ca:(c + 1) * ca, :])
                pre["w"] = (w1t, w2t)
            if "nog1" in ablate:
                ht_lo = ht_pool.tile([P, NT1 // 2, TH], bf16, name="ht_lo")
                ht_hi = ht_pool.tile([P, NT1 // 2, TH], bf16, name="ht_hi")
                nc.gpsimd.memset(ht_lo[:], 0.25)
                nc.gpsimd.memset(ht_hi[:], 0.25)
                pre["ht"] = (ht_lo, ht_hi)
            for _rep in range(repeats):
                _emit_rep(nc, f32, bf16, gelu, x_ap, out_ap, w1r, w2r,
                          pools, ablate, pre)
    return nc


def _emit_rep(nc, f32, bf16, gelu, x_ap, out_ap, w1r, w2r, pools,
              ablate=frozenset(), pre=None):
    (w1_pool, w2_pool, xrow_pool, xt_pool, ht_pool, ost_pool,
     hpsum_pool, opsum_pool) = pools
    pre = pre or {}

    # ---- resident weight tiles (reloaded each rep) --------------------
    if "w" in pre:
        w1t, w2t = pre["w"]
    else:
        w1t = w1_pool.tile([P, KT, INT], bf16, name="w1t")
        w2t = w2_pool.tile([P, NT1, OUT], bf16, name="w2t")

    # ---- gpsimd (SWDGE casting-DMA) queue, in hand-scheduled order ----
    # x casts for each stripe are interleaved between weight chunks so a
    # stripe's x is converted well before its transposes need it, while w1
    # still gets the bulk of early queue time (stripe 0 consumes it first).
    xrows = {}  # stripe -> list of 4 bf16 [P, HID] tiles

    def emit_x_casts(s):
        ts = []
        t0 = s * TH
        for r in range(RT):
            xr = xrow_pool.tile([P, HID], bf16, name="xr")
            nc.gpsimd.dma_start(
                out=xr[:], in_=x_ap[t0 + r * P:t0 + (r + 1) * P, :]
            )
            ts.append(xr)
        xrows[s] = ts

    def emit_w1_chunks(lo, hi):
        cw = INT // W1C
        for c in range(lo, hi):
            nc.gpsimd.dma_start(
                out=w1t[:, :, c * cw:(c + 1) * cw],
                in_=w1r[:, :, c * cw:(c + 1) * cw],
            )

    def emit_w2_chunks(lo, hi):
        ca = NT1 // W2C
        for c in range(lo, hi):
            nc.gpsimd.dma_start(
                out=w2t[:, c * ca:(c + 1) * ca, :],
                in_=w2r[:, c * ca:(c + 1) * ca, :],
            )

    skip_x = "nog1" in ablate
    skip_w = "w" in pre
    if not skip_x:
        emit_x_casts(0)
    if not skip_w:
        emit_w1_chunks(0, 8)
    if not skip_x:
        emit_x_casts(1)
    if not skip_w:
        emit_w1_chunks(8, 16)
        emit_w2_chunks(0, 4)
    if not skip_x:
        emit_x_casts(2)
    if not skip_w:
        emit_w2_chunks(4, 10)
    if not skip_x:
        emit_x_casts(3)
    if not skip_w:
        emit_w2_chunks(10, 16)

    # ---- x transpose via DMA XBAR (sync queue) ------------------------
    def emit_xt(s):
        xt = xt_pool.tile([P, KT, TH], bf16, name="xt")
        for r in range(RT):
            nc.sync.dma_start(
                out=xt[:, :, r * P:(r + 1) * P],
                in_=xrows[s][r][:],
                transpose=True,
            )
        return xt

    xt_s = emit_xt(0) if not skip_x else None

    last_hp = None
    for s in range(NS):
        # ---------------- GEMM1: hT[n, tok] = w1.T @ xT ----------------
        if "nog1" in ablate:
            ht_lo, ht_hi = pre["ht"]
        else:
            ht_lo = ht_pool.tile([P, NT1 // 2, TH], bf16, name="ht_lo")
            ht_hi = ht_pool.tile([P, NT1 // 2, TH], bf16, name="ht_hi")

            hp = None
            for j in range(NT1):
                if j % 2 == 0:
                    hp = hpsum_pool.tile([P, 2, TH], f32, name="hp")
                for k in range(KT):
                    nc.tensor.matmul(
                        hp[:, j % 2, :],
                        w1t[:, k, j * P:(j + 1) * P],
                        xt_s[:, k, :],
                        start=(k == 0),
                        stop=(k == KT - 1),
                    )
                if j % 2 == 1:
                    htt = ht_lo if j < NT1 // 2 else ht_hi
                    jj = j if j < NT1 // 2 else j - NT1 // 2
                    nc.scalar.activation(
                        htt[:, jj - 1:jj + 1, :], hp[:, :, :], gelu
                    )
            last_hp = hp

        # ------------- GEMM2: out[tok, d] = hT.T @ w2 ------------------
        # one 32-deep PSUM accumulation chain per (t, d); eviction via ACT
        # copy to SBUF staging, then DMA out.
        t0 = s * TH
        if "nog2" in ablate:
            if s == NS - 1:
                ot = ost_pool.tile([P, 512], f32, name="ot")
                nc.scalar.copy(ot[:], last_hp[:, 0, :])
                nc.sync.dma_start(out=out_ap[0:P, 0:512], in_=ot[:])
            if not skip_x and s + 1 < NS:
                xt_s = emit_xt(s + 1)
            continue
        for t in range(TT):
            for d in range(ND):
                po = opsum_pool.tile([P, 512], f32, name="po")
                for j in range(NT1):
                    htt = ht_lo if j < NT1 // 2 else ht_hi
                    jj = j if j < NT1 // 2 else j - NT1 // 2
                    nc.tensor.matmul(
                        po[:],
                        htt[:, jj, t * P:(t + 1) * P],
                        w2t[:, j, d * 512:(d + 1) * 512],
                        start=(j == 0),
                        stop=(j == NT1 - 1),
                    )
                if "noout" in ablate and s != NS - 1:
                    continue
                ot = ost_pool.tile([P, 512], f32, name="ot")
                nc.scalar.copy(ot[:], po[:])
                nc.sync.dma_start(
                    out=out_ap[t0 + t * P:t0 + (t + 1) * P,
                               d * 512:(d + 1) * 512],
                    in_=ot[:],
                )
            if t == 0 and not skip_x and s + 1 < NS:
                xt_s = emit_xt(s + 1)


def _get_program():
    if "v3" not in _PROGRAM_CACHE:
        _PROGRAM_CACHE["v3"] = build_program()
    return _PROGRAM_CACHE["v3"]


def kernel(x, w1, w2, _trace=False, _trace_kwargs=None):
    """Full-tensor entry point: shards experts across 8 cores, returns full out."""
    from concourse.bass_utils import run_bass_kernel_spmd

    _install_wait_split_hook()
    x = np.ascontiguousarray(x, dtype=np.float32)
    w1 = np.ascontiguousarray(w1, dtype=np.float32)
    w2 = np.ascontiguousarray(w2, dtype=np.float32)
    assert x.shape == (NUM_EXPERTS, TOK, HID)
    assert w1.shape == (NUM_EXPERTS, HID, INT)
    assert w2.shape == (NUM_EXPERTS, INT, OUT)

    nc = _get_program()
    core_ids = list(range(NUM_EXPERTS))
    in_maps = [
        {"x": x[e], "w1": w1[e], "w2": w2[e]} for e in range(NUM_EXPERTS)
    ]
    kw = {}
    if _trace:
        kw["trace"] = True
        kw["trace_kwargs"] = _trace_kwargs or {}
    res = run_bass_kernel_spmd(nc, in_maps, core_ids, **kw)
    out = np.stack([res.results[e]["out"] for e in range(NUM_EXPERTS)], axis=0)
    if _trace:
        return out, res
    return out


if __name__ == "__main__":
    rng = np.random.default_rng(0)
    x = rng.standard_normal((NUM_EXPERTS, TOK, HID), dtype=np.float32)
    w1 = rng.standard_normal((NUM_EXPERTS, HID, INT), dtype=np.float32) * 0.03
    w2 = rng.standard_normal((NUM_EXPERTS, INT, OUT), dtype=np.float32) * 0.015
    out = kernel(x, w1, w2)
    print("out", out.shape, out.dtype, float(np.abs(out).mean()))
